# revision 19
# baseline (speedup 1.0000x reference)
"""Multi-head causal attention (B=2, S=2048, D=1024, H=16, hd=64) on 8 TRN2
NeuronCores.

Sharding: tensor-parallel over heads - 2 heads per core. Each core computes
Q/K/V for its 2 heads over the full sequence, causal attention, and a partial
output projection (its 128 context features x Wo slice). Host sums the 8
partials (f16) and adds the bias.

v4 design (all f16 matmuls, fp32 PSUM):
  - Scores matmuls (K=hd=64) run as row-tiled head pairs: h0 on PE rows
    0-63, h1 on rows 64-127 (concurrent via auto tile_position from the
    operands' base partitions). ~1.35x over serial heads (bus arbitration
    limits the ideal 2x).
  - One exp (ACT) per key chunk covering both heads ([128, 2, N] psum
    tile spanning 2 banks): halves ACT instruction count vs per-head.
  - ctx matmuls M=65 (V + ones column for free row sums), serial heads;
    moving-operand bus is fully utilized so this is already at roofline.
  - Per-block softmax denominators: rowsum row extract + K=1 matmul
    broadcast + DVE reciprocal all in the block TAIL; only the two fused
    normalize-multiplies sit on the next block's critical path.
  - QKV projection interleaved chunk-by-chunk with attention (the
    ACT-bound attention loop overlaps the PE-bound projections), deferred
    out-projection slabs as additional PE filler.
  - Bulk x prefetch spread across the scalar and gpsimd DGE queues so the
    latency-critical sync-queue DMAs (first x block, V transposes, output
    slabs) are never stuck behind megabytes of prefetch.
"""
import sys
from collections import deque

for _p in ("/opt/trn_rl_repo",):
    if _p not in sys.path:
        sys.path.insert(0, _p)

import numpy as np

import concourse.bass as bass
import concourse.mybir as mybir
import concourse.tile as tile
from concourse import bacc
from concourse.bass_utils import run_bass_kernel_spmd

B, S, D = 2, 2048, 1024
H, HD = 16, 64
T = B * S                      # 4096 tokens
NCORES = 8
HPC = H // NCORES              # heads per core = 2
CF = HPC * HD                  # per-core ctx features = 128
QBLK = 512                     # query block width
NQB = S // QBLK                # 4 query blocks per batch
KCH = 128                      # key chunk
NFC = D // 128                 # contraction chunks for the projections
NTB = T // 1024                # 1024-token x blocks
F16 = mybir.dt.float16
F32 = mybir.dt.float32
F32R = mybir.dt.float32r
AF = mybir.ActivationFunctionType
MUL = mybir.AluOpType.mult


def build_kernel():
    nc = bacc.Bacc()
    xT = nc.dram_tensor("xT", [128, NTB, NFC, 1024], F16, kind="ExternalInput")
    wq = nc.dram_tensor("wq", [128, NFC, 128], F16, kind="ExternalInput")
    wk = nc.dram_tensor("wk", [128, NFC, 128], F16, kind="ExternalInput")
    wv = nc.dram_tensor("wv", [128, NFC, 128], F16, kind="ExternalInput")
    wo = nc.dram_tensor("wo", [CF, D], F16, kind="ExternalInput")
    tri = nc.dram_tensor("tri", [128, 2, 128], F16, kind="ExternalInput")
    ind2 = nc.dram_tensor("ind2", [2, 128], F16, kind="ExternalInput")
    part = nc.dram_tensor("part", [T, D], F16, kind="ExternalOutput")

    with tile.TileContext(nc) as tc:
        with (
            tc.tile_pool(name="persist", bufs=1) as persist,
            tc.tile_pool(name="qkv_sb", bufs=1) as qkv_sb,
        ):
            # ---- weights / constants ----
            wq_sb = persist.tile([128, NFC, 128], F16, tag="wq")
            wk_sb = persist.tile([128, NFC, 128], F16, tag="wk")
            wv_sb = persist.tile([128, NFC, 128], F16, tag="wv")
            wo_sb = persist.tile([128, D], F16, tag="wo")
            tri_sb = persist.tile([128, 2, 128], F16, tag="tri")
            indA_sb = persist.tile([1, 128], F16, tag="indA")
            indB_sb = persist.tile([1, 128], F16, tag="indB")
            ind_sbs = [indA_sb, indB_sb]
            warm_in = persist.tile([1, 16], F32, tag="wi")
            warm_out = persist.tile([1, 16], F16, tag="wo2")

            # exp table load happens during the projection phase
            nc.gpsimd.memset(warm_in[:, :], 0.0)
            nc.scalar.activation(warm_out[:, :], warm_in[:, :], AF.Exp,
                                 bias=0.0, scale=0.125)

            nc.scalar.dma_start(wq_sb[:, :, :], wq[:, :, :])
            nc.scalar.dma_start(wk_sb[:, :, :], wk[:, :, :])
            nc.scalar.dma_start(wv_sb[:, :, :], wv[:, :, :])

            # ---- persistent activations ----
            qt_sb = qkv_sb.tile([128, T], F16, tag="qt")
            kt_sb = qkv_sb.tile([128, T], F16, tag="kt")
            vt_sb = qkv_sb.tile([128, T], F16, tag="vt")

            with (
                tc.tile_pool(name="xp", bufs=NTB) as xp,
                tc.tile_pool(name="sps_ps", bufs=2, space="PSUM") as sps_ps,
                tc.tile_pool(name="ctx_ps", bufs=1, space="PSUM") as ctx_ps,
                tc.tile_pool(name="mm_ps", bufs=2, space="PSUM") as mm_ps,
                tc.tile_pool(name="probs", bufs=4) as probs_pool,
                tc.tile_pool(name="vstage", bufs=2) as vstage_pool,
                tc.tile_pool(name="vpp", bufs=6) as vp_pool,
                tc.tile_pool(name="normp", bufs=3) as norm_pool,
                tc.tile_pool(name="rrp", bufs=4) as rr_pool,
                tc.tile_pool(name="recp", bufs=2) as rec_pool,
                tc.tile_pool(name="outp", bufs=4) as out_pool,
            ):
                # ---- x DMAs: ONLY tb0 (split across all 3 DGE rings) plus
                # the small constants upfront; xt1-3 are gated on vt(tb0)
                # via 1-element copies so the scheduler cannot hoist their
                # transfers into the startup-critical window ----
                xts = []
                for tb in range(NTB):
                    xt = xp.tile([128, NFC, 1024], F16, tag="x")
                    if tb == 0:
                        nc.sync.dma_start(xt[:, 0:3, :], xT[:, 0, 0:3, :])
                        nc.gpsimd.dma_start(xt[:, 3:6, :], xT[:, 0, 3:6, :])
                        nc.scalar.dma_start(xt[:, 6:8, :], xT[:, 0, 6:8, :])
                    xts.append(xt)
                nc.scalar.dma_start(wo_sb[:, :], wo[:, :])
                nc.scalar.dma_start(tri_sb[:, :, :], tri[:, :, :])
                nc.scalar.dma_start(indA_sb[:, :], ind2[0:1, :])
                nc.scalar.dma_start(indB_sb[:, :], ind2[1:2, :])

                def prefetch_rest():
                    # RAW on vt_sb(tb0) + WAW with the big DMA = real gate
                    for tb, eng in ((1, nc.scalar), (2, nc.gpsimd),
                                    (3, nc.sync)):
                        nc.vector.tensor_copy(xts[tb][0:1, 0:1, 0:1],
                                              vt_sb[0:1, 1023:1024])
                        eng.dma_start(xts[tb][:, :, :], xT[:, tb, :, :])

                vp_map = {}

                def qkv_group(tb, w_sb, dst, s0):
                    def emit():
                        ps = mm_ps.tile([128, 512], F32, tag="mm")
                        xt = xts[tb]
                        for f in range(NFC):
                            nc.tensor.matmul(
                                ps[:, :], w_sb[:, f, :], xt[:, f, s0:s0 + 512],
                                start=(f == 0), stop=(f == NFC - 1),
                            )
                        nc.vector.tensor_copy(
                            dst[:, tb * 1024 + s0: tb * 1024 + s0 + 512],
                            ps[:, :],
                        )
                    return emit

                def qkv_groups(tb):
                    return [
                        qkv_group(tb, wq_sb, qt_sb, 0),
                        qkv_group(tb, wk_sb, kt_sb, 0),
                        qkv_group(tb, wq_sb, qt_sb, 512),
                        qkv_group(tb, wk_sb, kt_sb, 512),
                        qkv_group(tb, wv_sb, vt_sb, 0),
                        qkv_group(tb, wv_sb, vt_sb, 512),
                    ]

                def vprep(tb):
                    def emit():
                        for h in range(HPC):
                            hp = slice(h * HD, (h + 1) * HD)
                            vstage = vstage_pool.tile([128, 8, HD], F16, tag="vs")
                            nc.sync.dma_start_transpose(
                                vstage[:, :, :],
                                vt_sb[hp, tb * 1024:(tb + 1) * 1024],
                            )
                            vp = vp_pool.tile([128, 8, HD + 1], F16, tag="vp")
                            nc.gpsimd.memset(vp[:, :, HD:HD + 1], 1.0)
                            nc.vector.tensor_copy(vp[:, :, 0:HD], vstage[:, :, :])
                            vp_map[(tb, h)] = vp
                    return emit

                # state threaded between attention blocks
                pend_norm = [None]   # (cps pair, recf, cell for ctx_sb)
                pend_out = [None]    # (cell, toff, q0)
                dma_alt = [0]

                def block_tail(cps):
                    """rowsum extract + K=1 matmul broadcast + reciprocal,
                    off the next block's critical path."""
                    rrA = rr_pool.tile([1, QBLK], F16, tag="rr")
                    nc.vector.tensor_copy(rrA[:, :], cps[0][HD:HD + 1, :])
                    rrB = rr_pool.tile([1, QBLK], F16, tag="rr")
                    nc.vector.tensor_copy(rrB[:, :], cps[1][HD:HD + 1, :])
                    recb = mm_ps.tile([128, QBLK], F32, tag="mm")
                    for h, rr in ((0, rrA), (1, rrB)):
                        nc.tensor.matmul(
                            recb[:, :], ind_sbs[h][:, :], rr[:, :],
                            start=(h == 0), stop=(h == HPC - 1),
                        )
                    recf = rec_pool.tile([128, QBLK], F32, tag="recf")
                    nc.vector.reciprocal_approx_fast(recf[:, :], recb[:, :])
                    return recf

                def emit_norm():
                    cps, recf, cell = pend_norm[0]
                    pend_norm[0] = None
                    ctx_sb = norm_pool.tile([128, QBLK], F16, tag="ctx")
                    # fused psum-read + normalize + f16 cast
                    nc.vector.tensor_tensor(
                        ctx_sb[0:HD, :], cps[0][0:HD, :], recf[0:HD, :], MUL)
                    nc.vector.tensor_tensor(
                        ctx_sb[HD:128, :], cps[1][0:HD, :], recf[HD:128, :], MUL)
                    cell[0] = ctx_sb

                def outproj_half(tch, s0, copy_eng="v"):
                    def emit():
                        cell, toff, q0 = pend_out[0]
                        ctx_sb = cell[0]
                        ops = mm_ps.tile([128, 512], F32, tag="mm")
                        nc.tensor.matmul(
                            ops[:, :],
                            ctx_sb[:, tch * 128:(tch + 1) * 128],
                            wo_sb[:, s0:s0 + 512],
                            start=True, stop=True,
                        )
                        osb = out_pool.tile([128, 512], F16, tag="o")
                        if copy_eng == "s":
                            nc.scalar.copy(osb[:, :], ops[:, :])
                        else:
                            nc.vector.tensor_copy(osb[:, :], ops[:, :])
                        t0 = toff + q0 + tch * 128
                        if dma_alt[0] % 2 == 0:
                            nc.sync.dma_start(part[t0:t0 + 128, s0:s0 + 512],
                                              osb[:, :])
                        else:
                            nc.gpsimd.dma_start(part[t0:t0 + 128, s0:s0 + 512],
                                                osb[:, :])
                        dma_alt[0] += 1
                    return emit

                def emit_ctx(b, cps, pend_chunk, nk):
                    probs, c, off = pend_chunk
                    tbv = b * 2 + (c * 128) // 1024
                    for h in range(HPC):
                        src = vp_map[(tbv, h)][:, (c * 128 % 1024) // 128, :]
                        nc.tensor.matmul(
                            cps[h][:, off:QBLK], src, probs[:, h, off:QBLK],
                            start=(c == 0), stop=(c == nk - 1),
                        )

                def att_block(b, qb, fillers):
                    toff, q0 = b * S, qb * QBLK
                    nk = (q0 + QBLK) // KCH
                    cps0 = ctx_ps.tile([HD + 1, QBLK], F32, tag="c0")
                    cps1 = ctx_ps.tile([HD + 1, QBLK], F32, tag="c1")
                    cps = [cps0, cps1]
                    pend_chunk = None
                    fq = deque(fillers)
                    for c in range(nk):
                        off = max(0, 128 * c - q0)
                        sps = sps_ps.tile([128, 2, QBLK], F32, tag="sps")
                        for h in range(HPC):
                            hp = slice(h * HD, (h + 1) * HD)
                            nc.tensor.matmul(
                                sps[:, h, off:QBLK],
                                kt_sb[hp, toff + c * 128: toff + (c + 1) * 128],
                                qt_sb[hp, toff + q0 + off: toff + q0 + QBLK],
                                start=True, stop=True,
                            )
                        probs = probs_pool.tile([128, 2, QBLK], F16, tag="p")
                        nc.scalar.activation(
                            probs[:, :, off:], sps[:, :, off:], AF.Exp,
                            bias=0.0, scale=0.125,
                        )
                        if c * 128 >= q0:
                            nc.gpsimd.tensor_tensor(
                                probs[:, :, off:off + 128],
                                probs[:, :, off:off + 128],
                                tri_sb[:, :, :], MUL,
                            )
                        if c == 0:
                            # previous block's normalize: frees its ctx psum
                            # accumulators before this block's first ctx mm
                            if pend_norm[0] is not None:
                                emit_norm()
                        else:
                            if pend_chunk is not None:
                                emit_ctx(b, cps, pend_chunk, nk)
                            rem = nk - c
                            want = (len(fq) + rem - 1) // rem
                            for _ in range(min(want, len(fq))):
                                fq.popleft()()
                        pend_chunk = (probs, c, off)
                    emit_ctx(b, cps, pend_chunk, nk)
                    while fq:
                        fq.popleft()()
                    recf = block_tail(cps)
                    cell = [None]
                    pend_norm[0] = (cps, recf, cell)
                    pend_out[0] = (cell, toff, q0)

                # ================= emission =================
                g0 = qkv_groups(0)
                for g in (g0[0], g0[1], g0[4], g0[5]):   # q0, k0, v0, v512
                    g()
                vprep(0)()
                g0[2]()                                   # q512
                g0[3]()                                   # k512
                prefetch_rest()

                def outs():
                    return [outproj_half(tch, s0) for tch in range(QBLK // 128)
                            for s0 in (0, 512)]

                # fillers: all of QKV(tb+1)+vprep in block qbB (its x block
                # streams in during qbA); qbA carries only the deferred
                # out-projection slabs
                for tb in range(NTB):
                    b = tb // 2
                    qbA = 2 * (tb % 2)
                    qbB = qbA + 1
                    if tb < NTB - 1:
                        fillB = qkv_groups(tb + 1) + [vprep(tb + 1)]
                    else:
                        fillB = []
                    att_block(b, qbA, (outs() if pend_out[0] else []))
                    att_block(b, qbB, outs() + fillB)

                # trailing normalize + out-projection for the final block:
                # PSUM->SBUF casts alternate ACT/DVE (ACT is idle at the
                # tail) so the 8 slabs drain in parallel
                emit_norm()
                for tch in range(QBLK // 128):
                    for s0 in (0, 512):
                        outproj_half(tch, s0, copy_eng="sv"[(tch + s0 // 512) % 2])()
    nc.compile()
    return nc


_NC_CACHE = None


def _get_nc():
    global _NC_CACHE
    if _NC_CACHE is None:
        _NC_CACHE = build_kernel()
    return _NC_CACHE


def _warr(w):
    """[D, CF] torch-style slice -> partition-major [128, NFC, 128]."""
    return np.ascontiguousarray(
        w.astype(np.float16).reshape(NFC, 128, CF).transpose(1, 0, 2)
    )


def make_in_maps(x, Wq, Wk, Wv, Wo):
    xf = x.reshape(T, D).astype(np.float16)
    xT = np.ascontiguousarray(
        xf.reshape(NTB, 1024, NFC, 128).transpose(3, 0, 2, 1)
    )
    tri1 = np.triu(np.ones((128, 128), dtype=np.float16))
    tri = np.ascontiguousarray(np.stack([tri1, tri1], axis=1))
    ind2 = np.zeros((2, 128), dtype=np.float16)
    ind2[0, 0:64] = 1.0
    ind2[1, 64:128] = 1.0
    in_maps = []
    for c in range(NCORES):
        rs = slice(c * CF, (c + 1) * CF)
        in_maps.append({
            "xT": xT,
            "wq": _warr(Wq[rs, :].T),
            "wk": _warr(Wk[rs, :].T),
            "wv": _warr(Wv[rs, :].T),
            "wo": np.ascontiguousarray(Wo[:, rs].T.astype(np.float16)),
            "tri": tri,
            "ind2": ind2,
        })
    return in_maps


def kernel(x, Wq, Wk, Wv, Wo, bo):
    x = np.asarray(x, dtype=np.float32)
    Wq = np.asarray(Wq, dtype=np.float32)
    Wk = np.asarray(Wk, dtype=np.float32)
    Wv = np.asarray(Wv, dtype=np.float32)
    Wo = np.asarray(Wo, dtype=np.float32)
    bo = np.asarray(bo, dtype=np.float32)

    in_maps = make_in_maps(x, Wq, Wk, Wv, Wo)
    res = run_bass_kernel_spmd(_get_nc(), in_maps, core_ids=list(range(NCORES)))
    out = res.results[0]["part"].astype(np.float32)
    for c in range(1, NCORES):
        out += res.results[c]["part"].astype(np.float32)
    out += bo[None, :]
    return out.reshape(B, S, D)


# revision 20
# speedup vs baseline: 1.2076x; 1.2076x over previous
"""Multi-head causal attention (B=2, S=2048, D=1024, H=16, hd=64) on 8 TRN2
NeuronCores.

Sharding: tensor-parallel over heads - 2 heads per core. Each core computes
Q/K/V for its 2 heads over the full sequence, causal attention, and a partial
output projection (its 128 context features x Wo slice). Host sums the 8
partials (f16) and adds the bias.

Design (all f16 matmuls, fp32 PSUM):
  - Scores matmuls (K=hd=64) run as row-tiled head pairs: h0 on PE rows
    0-63, h1 on rows 64-127 (concurrent via auto tile_position from the
    operands' base partitions) - measured ~227ns per warm pair at N=512,
    near the moving-operand bus roofline.
  - One exp (ACT) per key chunk covering both heads ([128, 2, N] psum
    tile spanning 2 banks): halves ACT instruction count vs per-head.
  - ctx matmuls M=65 (V + ones column for free row sums), serial heads;
    the moving-operand bus is fully utilized so this is at roofline.
  - Per-block softmax denominators: rowsum row extract + K=1 matmul
    broadcast + DVE reciprocal in the block TAIL; only the two fused
    normalize-multiplies sit on the next block's critical path.
  - QKV projection interleaved chunk-by-chunk with attention (the
    ACT-bound attention loop overlaps the PE-bound projections), deferred
    out-projection slabs as additional PE filler.
  - Bulk x prefetch spread across the scalar and gpsimd DGE queues so the
    sync queue's latency-critical DMAs (first x block, V transposes,
    output slabs) are never stuck behind megabytes of prefetch.
"""
import sys
from collections import deque

for _p in ("/opt/trn_rl_repo",):
    if _p not in sys.path:
        sys.path.insert(0, _p)

import numpy as np

import concourse.bass as bass
import concourse.mybir as mybir
import concourse.tile as tile
from concourse import bacc
from concourse.bass_utils import run_bass_kernel_spmd

B, S, D = 2, 2048, 1024
H, HD = 16, 64
T = B * S                      # 4096 tokens
NCORES = 8
HPC = H // NCORES              # heads per core = 2
CF = HPC * HD                  # per-core ctx features = 128
QBLK = 512                     # query block width
NQB = S // QBLK                # 4 query blocks per batch
KCH = 128                      # key chunk
NFC = D // 128                 # contraction chunks for the projections
NTB = T // 1024                # 1024-token x blocks
F16 = mybir.dt.float16
F32 = mybir.dt.float32
F32R = mybir.dt.float32r
AF = mybir.ActivationFunctionType
MUL = mybir.AluOpType.mult


def build_kernel():
    nc = bacc.Bacc()
    xT = nc.dram_tensor("xT", [128, NTB, NFC, 1024], F16, kind="ExternalInput")
    wq = nc.dram_tensor("wq", [128, NFC, 128], F16, kind="ExternalInput")
    wk = nc.dram_tensor("wk", [128, NFC, 128], F16, kind="ExternalInput")
    wv = nc.dram_tensor("wv", [128, NFC, 128], F16, kind="ExternalInput")
    wo = nc.dram_tensor("wo", [CF, D], F16, kind="ExternalInput")
    tri = nc.dram_tensor("tri", [128, 2, 128], F16, kind="ExternalInput")
    ind2 = nc.dram_tensor("ind2", [2, 128], F32R, kind="ExternalInput")
    part = nc.dram_tensor("part", [T, D], F16, kind="ExternalOutput")

    with tile.TileContext(nc) as tc:
        with (
            tc.tile_pool(name="persist", bufs=1) as persist,
            tc.tile_pool(name="qkv_sb", bufs=1) as qkv_sb,
        ):
            # ---- weights / constants ----
            wq_sb = persist.tile([128, NFC, 128], F16, tag="wq")
            wk_sb = persist.tile([128, NFC, 128], F16, tag="wk")
            wv_sb = persist.tile([128, NFC, 128], F16, tag="wv")
            wo_sb = persist.tile([128, D], F16, tag="wo")
            tri_sb = persist.tile([128, 2, 128], F16, tag="tri")
            indA_sb = persist.tile([1, 128], F32R, tag="indA")
            indB_sb = persist.tile([1, 128], F32R, tag="indB")
            ind_sbs = [indA_sb, indB_sb]
            warm_in = persist.tile([1, 16], F32, tag="wi")
            warm_out = persist.tile([1, 16], F16, tag="wo2")

            # exp table load happens during the projection phase
            nc.gpsimd.memset(warm_in[:, :], 0.0)
            nc.scalar.activation(warm_out[:, :], warm_in[:, :], AF.Exp,
                                 bias=0.0, scale=0.125)

            nc.scalar.dma_start(wq_sb[:, :, :], wq[:, :, :])
            nc.scalar.dma_start(wk_sb[:, :, :], wk[:, :, :])
            nc.scalar.dma_start(wv_sb[:, :, :], wv[:, :, :])

            # ---- persistent activations ----
            qt_sb = qkv_sb.tile([128, T], F16, tag="qt")
            kt_sb = qkv_sb.tile([128, T], F16, tag="kt")
            vt_sb = qkv_sb.tile([128, T], F16, tag="vt")

            with (
                tc.tile_pool(name="xp", bufs=NTB) as xp,
                tc.tile_pool(name="sps_ps", bufs=2, space="PSUM") as sps_ps,
                tc.tile_pool(name="ctx_ps", bufs=1, space="PSUM") as ctx_ps,
                tc.tile_pool(name="mm_ps", bufs=2, space="PSUM") as mm_ps,
                tc.tile_pool(name="probs", bufs=4) as probs_pool,
                tc.tile_pool(name="vstage", bufs=2) as vstage_pool,
                tc.tile_pool(name="vpp", bufs=6) as vp_pool,
                tc.tile_pool(name="normp", bufs=3) as norm_pool,
                tc.tile_pool(name="rrp", bufs=4) as rr_pool,
                tc.tile_pool(name="recp", bufs=2) as rec_pool,
                tc.tile_pool(name="outp", bufs=4) as out_pool,
            ):
                # ---- x DMAs: tb0 split sync/scalar (needed first); the
                # bulk prefetch rides the scalar + gpsimd queues so it
                # never delays the sync queue's transposes/output slabs ----
                xts = []
                for tb in range(NTB):
                    xt = xp.tile([128, NFC, 1024], F16, tag="x")
                    if tb == 0:
                        nc.sync.dma_start(xt[:, 0:4, :], xT[:, 0, 0:4, :])
                        nc.scalar.dma_start(xt[:, 4:8, :], xT[:, 0, 4:8, :])
                    elif tb == 1:
                        nc.scalar.dma_start(xt[:, :, :], xT[:, tb, :, :])
                    else:
                        nc.gpsimd.dma_start(xt[:, :, :], xT[:, tb, :, :])
                    xts.append(xt)
                nc.scalar.dma_start(wo_sb[:, :], wo[:, :])
                nc.scalar.dma_start(tri_sb[:, :, :], tri[:, :, :])
                nc.scalar.dma_start(indA_sb[:, :], ind2[0:1, :])
                nc.scalar.dma_start(indB_sb[:, :], ind2[1:2, :])

                vp_map = {}

                def qkv_group(tb, w_sb, dst, s0):
                    def emit():
                        ps = mm_ps.tile([128, 512], F32, tag="mm")
                        xt = xts[tb]
                        for f in range(NFC):
                            nc.tensor.matmul(
                                ps[:, :], w_sb[:, f, :], xt[:, f, s0:s0 + 512],
                                start=(f == 0), stop=(f == NFC - 1),
                            )
                        nc.vector.tensor_copy(
                            dst[:, tb * 1024 + s0: tb * 1024 + s0 + 512],
                            ps[:, :],
                        )
                    return emit

                def qkv_groups(tb):
                    return [
                        qkv_group(tb, wq_sb, qt_sb, 0),
                        qkv_group(tb, wk_sb, kt_sb, 0),
                        qkv_group(tb, wq_sb, qt_sb, 512),
                        qkv_group(tb, wk_sb, kt_sb, 512),
                        qkv_group(tb, wv_sb, vt_sb, 0),
                        qkv_group(tb, wv_sb, vt_sb, 512),
                    ]

                def vprep(tb):
                    def emit():
                        for h in range(HPC):
                            hp = slice(h * HD, (h + 1) * HD)
                            vstage = vstage_pool.tile([128, 8, HD], F16, tag="vs")
                            nc.sync.dma_start_transpose(
                                vstage[:, :, :],
                                vt_sb[hp, tb * 1024:(tb + 1) * 1024],
                            )
                            vp = vp_pool.tile([128, 8, HD + 1], F16, tag="vp")
                            nc.gpsimd.memset(vp[:, :, HD:HD + 1], 1.0)
                            nc.vector.tensor_copy(vp[:, :, 0:HD], vstage[:, :, :])
                            vp_map[(tb, h)] = vp
                    return emit

                # state threaded between attention blocks
                pend_norm = [None]   # (cps pair, recf, cell for ctx_sb)
                pend_out = [None]    # (cell, toff, q0)
                dma_alt = [0]

                def block_tail(cps):
                    """rowsum extract + K=1 matmul broadcast + reciprocal,
                    off the next block's critical path."""
                    rrA = rr_pool.tile([1, QBLK], F32R, tag="rr")
                    nc.vector.tensor_copy(rrA[:, :], cps[0][HD:HD + 1, :])
                    rrB = rr_pool.tile([1, QBLK], F32R, tag="rr")
                    nc.vector.tensor_copy(rrB[:, :], cps[1][HD:HD + 1, :])
                    recb = mm_ps.tile([128, QBLK], F32, tag="mm")
                    for h, rr in ((0, rrA), (1, rrB)):
                        nc.tensor.matmul(
                            recb[:, :], ind_sbs[h][:, :], rr[:, :],
                            start=(h == 0), stop=(h == HPC - 1),
                        )
                    recf = rec_pool.tile([128, QBLK], F32, tag="recf")
                    nc.vector.reciprocal_approx_fast(recf[:, :], recb[:, :])
                    return recf

                def emit_norm():
                    cps, recf, cell = pend_norm[0]
                    pend_norm[0] = None
                    ctx_sb = norm_pool.tile([128, QBLK], F16, tag="ctx")
                    # fused psum-read + normalize + f16 cast
                    nc.vector.tensor_tensor(
                        ctx_sb[0:HD, :], cps[0][0:HD, :], recf[0:HD, :], MUL)
                    nc.vector.tensor_tensor(
                        ctx_sb[HD:128, :], cps[1][0:HD, :], recf[HD:128, :], MUL)
                    cell[0] = ctx_sb

                def outproj_half(tch, s0):
                    def emit():
                        cell, toff, q0 = pend_out[0]
                        ctx_sb = cell[0]
                        ops = mm_ps.tile([128, 512], F32, tag="mm")
                        nc.tensor.matmul(
                            ops[:, :],
                            ctx_sb[:, tch * 128:(tch + 1) * 128],
                            wo_sb[:, s0:s0 + 512],
                            start=True, stop=True,
                        )
                        osb = out_pool.tile([128, 512], F16, tag="o")
                        nc.vector.tensor_copy(osb[:, :], ops[:, :])
                        t0 = toff + q0 + tch * 128
                        if dma_alt[0] % 2 == 0:
                            nc.sync.dma_start(part[t0:t0 + 128, s0:s0 + 512],
                                              osb[:, :])
                        else:
                            nc.gpsimd.dma_start(part[t0:t0 + 128, s0:s0 + 512],
                                                osb[:, :])
                        dma_alt[0] += 1
                    return emit

                def emit_ctx(b, cps, pend_chunk, nk):
                    probs, c, off = pend_chunk
                    tbv = b * 2 + (c * 128) // 1024
                    for h in range(HPC):
                        src = vp_map[(tbv, h)][:, (c * 128 % 1024) // 128, :]
                        nc.tensor.matmul(
                            cps[h][:, off:QBLK], src, probs[:, h, off:QBLK],
                            start=(c == 0), stop=(c == nk - 1),
                        )

                def att_block(b, qb, fillers):
                    toff, q0 = b * S, qb * QBLK
                    nk = (q0 + QBLK) // KCH
                    cps0 = ctx_ps.tile([HD + 1, QBLK], F32, tag="c0")
                    cps1 = ctx_ps.tile([HD + 1, QBLK], F32, tag="c1")
                    cps = [cps0, cps1]
                    pend_chunk = None
                    fq = deque(fillers)
                    for c in range(nk):
                        off = max(0, 128 * c - q0)
                        sps = sps_ps.tile([128, 2, QBLK], F32, tag="sps")
                        for h in range(HPC):
                            hp = slice(h * HD, (h + 1) * HD)
                            nc.tensor.matmul(
                                sps[:, h, off:QBLK],
                                kt_sb[hp, toff + c * 128: toff + (c + 1) * 128],
                                qt_sb[hp, toff + q0 + off: toff + q0 + QBLK],
                                start=True, stop=True,
                            )
                        probs = probs_pool.tile([128, 2, QBLK], F16, tag="p")
                        nc.scalar.activation(
                            probs[:, :, off:], sps[:, :, off:], AF.Exp,
                            bias=0.0, scale=0.125,
                        )
                        if c * 128 >= q0:
                            nc.gpsimd.tensor_tensor(
                                probs[:, :, off:off + 128],
                                probs[:, :, off:off + 128],
                                tri_sb[:, :, :], MUL,
                            )
                        if c == 0:
                            # previous block's normalize: frees its ctx psum
                            # accumulators before this block's first ctx mm
                            if pend_norm[0] is not None:
                                emit_norm()
                        else:
                            if pend_chunk is not None:
                                emit_ctx(b, cps, pend_chunk, nk)
                            rem = nk - c
                            want = (len(fq) + rem - 1) // rem
                            for _ in range(min(want, len(fq))):
                                fq.popleft()()
                        pend_chunk = (probs, c, off)
                    emit_ctx(b, cps, pend_chunk, nk)
                    while fq:
                        fq.popleft()()
                    recf = block_tail(cps)
                    cell = [None]
                    pend_norm[0] = (cps, recf, cell)
                    pend_out[0] = (cell, toff, q0)

                # ================= emission =================
                g0 = qkv_groups(0)
                for g in (g0[0], g0[1], g0[4], g0[5]):   # q0, k0, v0, v512
                    g()
                vprep(0)()
                g0[2]()                                   # q512
                g0[3]()                                   # k512

                def outs():
                    return [outproj_half(tch, s0) for tch in range(QBLK // 128)
                            for s0 in (0, 512)]

                # fillers per (tb, block): QKV(tb+1) groups land one tb early;
                # QKV(tb3)'s v512+vprep slide into att(tb3) qb2 (its chunks
                # 0-7 only touch tb2 keys, so the deadline is chunk 8)
                carry = []
                for tb in range(NTB):
                    b = tb // 2
                    qbA = 2 * (tb % 2)
                    qbB = qbA + 1
                    if tb < NTB - 1:
                        nxt = qkv_groups(tb + 1)
                        if tb == NTB - 2:
                            fillA, fillB = nxt[:2], nxt[2:5]
                            nextcarry = [nxt[5], vprep(tb + 1)]
                        else:
                            fillA, fillB = nxt[:2], nxt[2:] + [vprep(tb + 1)]
                            nextcarry = []
                    else:
                        fillA, fillB, nextcarry = [], [], []
                    att_block(b, qbA, carry +
                              (outs() if pend_out[0] else []) + fillA)
                    att_block(b, qbB, outs() + fillB)
                    carry = nextcarry

                # trailing normalize + out-projection for the final block
                emit_norm()
                for tch in range(QBLK // 128):
                    for s0 in (0, 512):
                        outproj_half(tch, s0)()
    nc.compile()
    return nc


_NC_CACHE = None


def _get_nc():
    global _NC_CACHE
    if _NC_CACHE is None:
        _NC_CACHE = build_kernel()
    return _NC_CACHE


def _warr(w):
    """[D, CF] torch-style slice -> partition-major [128, NFC, 128]."""
    return np.ascontiguousarray(
        w.astype(np.float16).reshape(NFC, 128, CF).transpose(1, 0, 2)
    )


def make_in_maps(x, Wq, Wk, Wv, Wo):
    xf = x.reshape(T, D).astype(np.float16)
    xT = np.ascontiguousarray(
        xf.reshape(NTB, 1024, NFC, 128).transpose(3, 0, 2, 1)
    )
    tri1 = np.triu(np.ones((128, 128), dtype=np.float16))
    tri = np.ascontiguousarray(np.stack([tri1, tri1], axis=1))
    ind2 = np.zeros((2, 128), dtype=np.float32)
    ind2[0, 0:64] = 1.0
    ind2[1, 64:128] = 1.0
    in_maps = []
    for c in range(NCORES):
        rs = slice(c * CF, (c + 1) * CF)
        in_maps.append({
            "xT": xT,
            "wq": _warr(Wq[rs, :].T),
            "wk": _warr(Wk[rs, :].T),
            "wv": _warr(Wv[rs, :].T),
            "wo": np.ascontiguousarray(Wo[:, rs].T.astype(np.float16)),
            "tri": tri,
            "ind2": ind2,
        })
    return in_maps


def kernel(x, Wq, Wk, Wv, Wo, bo):
    x = np.asarray(x, dtype=np.float32)
    Wq = np.asarray(Wq, dtype=np.float32)
    Wk = np.asarray(Wk, dtype=np.float32)
    Wv = np.asarray(Wv, dtype=np.float32)
    Wo = np.asarray(Wo, dtype=np.float32)
    bo = np.asarray(bo, dtype=np.float32)

    in_maps = make_in_maps(x, Wq, Wk, Wv, Wo)
    res = run_bass_kernel_spmd(_get_nc(), in_maps, core_ids=list(range(NCORES)))
    out = res.results[0]["part"].astype(np.float32)
    for c in range(1, NCORES):
        out += res.results[c]["part"].astype(np.float32)
    out += bo[None, :]
    return out.reshape(B, S, D)
